# revision 2
# baseline (speedup 1.0000x reference)
"""LIF spiking-neuron kernel for Trainium2 (8 NeuronCores, data-parallel).

Problem: x [256,128,32,32] f32 viewed per core as [T=4, C=128, FREE=8192]
(batch dim B=64 split 8 ways, C on SBUF partitions); per-element recurrence
    u_t = m_{t-1}*0.5 + x_t ; spike_t = (u_t >= 1) ; m_t = (1-spike_t)*u_t

Engine split (f32 recurrence -> matches the reference bitwise except at
measure-zero u_t == 1.0 points, where the packed byte can flip one bit):
  - DVE: the two scalar_tensor_tensor ops per step (integrate in-place,
    hard reset (u<1)*u) -- the 24 STT ops (~49us busy) are the compute
    floor; every cheaper reformulation (min/Relu identities, scans,
    predicated copies, PE/PSUM offload, SWDGE accumulate-loads) measured
    slower or required 2-tensor combines DVE must do anyway.
  - ACT: spike = Sign(1-u) -> bf16 {+1,0,-1} (+1 = "no spike").
  - PE (otherwise idle): output BIT-PACKING. w[128,16] bf16 block-diagonal
    powers-of-2 (w[p, p//8] = 2^(p%8)); 4 matmuls per chunk-step write
    [16,512] slices at PSUM partition bases 0/32/64/96 (explicit
    tile_position) of a 2-bank [128,1024] image shared by 2 chunk-steps.
  - ACT: ONE decode per image: u8(-0.5*psum + 127.5) == packed spike byte
    (exact: psum = sum_b 2^b*sgn_b, so (255-psum)/2 = sum_b 2^b*spike_b).
  - output DMAs ride SWDGE (Pool engine) so neither HWDGE ring nor the
    ACT sequencer carries them. Output traffic 1 MiB/core vs 4 MiB for
    u8 spikes, 16 MiB for f32.

DMA: the input stream is split across BOTH HWDGE rings (SP + ACT) --
one ring alone caps at ~355 GB/s and finishes only as the last chunk's
recurrence needs it (stalling DVE); two rings reach ~550 GB/s so input
is fully hidden under compute. HW-measured (8-core, repeat-subtraction):
one-ring 49.7us, two-ring 32.1us for the 16 MiB input; this kernel
63.4us/iter vs 78.9us for the unpacked one-ring baseline.
"""

import contextlib

import numpy as np

import concourse.bass as bass
import concourse.tile as tile
from concourse import bacc, mybir
from concourse.bass_utils import run_bass_kernel_spmd

T = 4
B = 64
C = 128
HW = 1024
N_CORES = 8
B_SH = B // N_CORES          # 8 batches per core
FREE = B_SH * HW             # 8192 free columns per timestep per core
F = 2048                     # chunk width; 4 chunks per core
NCHUNK = FREE // F
NPAIR = (NCHUNK * T) // 2    # 8 packed output images per core
TAU = 0.5
THRESH = 1.0
LAG = 2                      # decode lag (pairs) behind the sign stream

_CACHED_NC = None
LAST_RESULTS = None          # exposed for test.py


def _pack_weights() -> np.ndarray:
    w = np.zeros((C, 16), dtype=np.float32)
    for p in range(C):
        w[p, p // 8] = float(2 ** (p % 8))
    return w.astype(mybir.dt.np(mybir.dt.bfloat16))


def _build_nc(reps: int = 1):
    """Per-core Bass program.  reps>1 wraps the body in a hardware loop
    (used only for repeat-subtraction wall-clock timing)."""
    f32 = mybir.dt.float32
    bf16 = mybir.dt.bfloat16
    u8 = mybir.dt.uint8
    op = mybir.AluOpType
    act = mybir.ActivationFunctionType

    nc = bacc.Bacc("TRN2", target_bir_lowering=False, debug=False)
    x = nc.dram_tensor("x", [T, C, FREE], f32, kind="ExternalInput").ap()
    # packed output: pair p holds steps (ci=p//2, t=2*(p%2)) in cols 0:512
    # and (ci=p//2, t=2*(p%2)+1) in cols 512:1024; within a 512-col half,
    # rows 32k..32k+16 are slice k (sgn cols 512k..512k+512); rows 16..31
    # of each 32-block are PSUM garbage the host ignores.
    o = nc.dram_tensor("o", [NPAIR, C, 1024], u8, kind="ExternalOutput").ap()
    wconst = nc.inline_tensor(_pack_weights(), name="wpack")

    with tile.TileContext(nc) as tc:
        with (
            tc.tile_pool(name="xs", bufs=10) as xpool,
            tc.tile_pool(name="ms", bufs=4) as mpool,
            tc.tile_pool(name="sg", bufs=6) as spool,
            tc.tile_pool(name="pk", bufs=3) as kpool,
            tc.tile_pool(name="wp", bufs=1) as wpool,
            tc.tile_pool(name="ps", bufs=3, space="PSUM") as ppool,
        ):
            wtile = wpool.tile([C, 16], bf16, name="w")
            nc.sync.dma_start(out=wtile[:], in_=wconst.ap())

            loop = tc.For_i(0, reps, 1) if reps > 1 else contextlib.nullcontext()
            with loop:
                pending = []   # [(pair_idx, psum_tile)]

                def flush_one():
                    p, pt = pending.pop(0)
                    packed = kpool.tile([C, 1024], u8, name=f"k_{p}", tag="k")
                    nc.scalar.activation(
                        packed[:], pt[:], act.Copy, bias=127.5, scale=-0.5,
                    )
                    nc.gpsimd.dma_start(out=o[p], in_=packed[:])

                step = 0
                pt = None
                for ci in range(NCHUNK):
                    xt = []
                    for t in range(T):
                        xtile = xpool.tile([C, F], f32, name=f"x_{ci}_{t}", tag="x")
                        eng = nc.sync if t % 2 == 0 else nc.scalar
                        eng.dma_start(out=xtile[:], in_=x[t, :, bass.ts(ci, F)])
                        xt.append(xtile)
                    m = None
                    for t in range(T):
                        u = xt[t]
                        if t > 0:
                            # u = m*tau + x_t (in-place; one f32 rounding,
                            # same as the reference)
                            nc.vector.scalar_tensor_tensor(
                                u[:], m[:], TAU, u[:], op.mult, op.add
                            )
                        sgn = spool.tile([C, F], bf16, name=f"s_{ci}_{t}", tag="s")
                        # Sign(1-u): +1 = no spike (u<1), 0/-1 = spike.
                        nc.scalar.activation(
                            sgn[:], u[:], act.Sign,
                            bias=float(THRESH), scale=-1.0,
                        )
                        if t < T - 1:
                            # m = (u < 1) * u  (hard reset)
                            mnew = mpool.tile([C, F], f32, name=f"m_{ci}_{t}", tag="m")
                            nc.vector.scalar_tensor_tensor(
                                mnew[:], u[:], THRESH, u[:], op.is_lt, op.mult
                            )
                            m = mnew
                        half = step % 2
                        if half == 0:
                            pt = ppool.tile([C, 1024], f32, name=f"p_{step // 2}", tag="p")
                        for k in range(4):
                            nc.tensor.matmul(
                                pt[32 * k:32 * k + 16,
                                   512 * half:512 * half + 512],
                                wtile[:], sgn[:, bass.ts(k, 512)],
                                start=True, stop=True,
                                tile_position=(0, 32 * k),
                            )
                        if half == 1:
                            pending.append((step // 2, pt))
                            if len(pending) > LAG:
                                flush_one()
                        step += 1
                while pending:
                    flush_one()

    nc.compile()
    return nc


def _decode_packed(o: np.ndarray) -> np.ndarray:
    """[NPAIR, 128, 1024] u8 pair images -> [T, C, FREE] f32 spikes."""
    o = o.reshape(NCHUNK, 2, C, 2, 512)            # [ci, tpair, row, half, f']
    banks = o.transpose(0, 1, 3, 2, 4).reshape(NCHUNK, T, 4, 32, 512)[:, :, :, :16, :]
    bits = np.unpackbits(banks[..., None], axis=-1, bitorder="little")
    # bits: [ci, t, k, j, f', b]; channel = 8j + b; free = ci*F + 512k + f'
    bits = bits.transpose(1, 3, 5, 0, 2, 4)
    return bits.reshape(T, C, FREE).astype(np.float32)


def kernel(x: np.ndarray) -> np.ndarray:
    global _CACHED_NC, LAST_RESULTS
    if _CACHED_NC is None:
        _CACHED_NC = _build_nc()
    nc = _CACHED_NC

    xs = np.ascontiguousarray(x, dtype=np.float32).reshape(T, B, C, HW)
    in_maps = []
    for mcore in range(N_CORES):
        shard = xs[:, mcore * B_SH:(mcore + 1) * B_SH]          # [T,B_sh,C,HW]
        shard = np.ascontiguousarray(shard.transpose(0, 2, 1, 3))  # [T,C,B_sh,HW]
        in_maps.append({"x": shard.reshape(T, C, FREE)})

    res = run_bass_kernel_spmd(nc, in_maps, list(range(N_CORES)))
    LAST_RESULTS = res

    outs = []
    for mcore in range(N_CORES):
        o = np.asarray(res.results[mcore]["o"])                 # [NPAIR,128,1024] u8
        sp = _decode_packed(o)                                  # [T,C,FREE]
        sp = sp.reshape(T, C, B_SH, HW).transpose(0, 2, 1, 3)   # [T,B_sh,C,HW]
        outs.append(sp)
    out = np.concatenate(outs, axis=1)                          # [T,B,C,HW]
    return np.ascontiguousarray(out.reshape(x.shape), dtype=np.float32)
